# revision 7
# baseline (speedup 1.0000x reference)
"""Trainium2 Bass kernel for AtomGNN (gnn_message_passing).

Strategy:
  - segment_sum is linear => commute msg-MLP layer 2 past the aggregation:
        m_sum[n] = sum_{e->n} relu(A[src_e] + B[dst_e] + zef_e) @ W2 + deg[n]*b2
    with A = h @ W1a, B = h @ W1b (node-level), zef = ef @ W1c + b1 (host).
  - Shard edges by dst node range across 8 cores: aggregation is fully local.
    Replicate A via AllGather (1.6MB/rank) once per round.
  - Edge phase per 4096-edge block: zef DMA (fp16->f32 cast), indirect-gather
    A[src] with CCE add, indirect-gather B[dst] with CCE add, DVE relu,
    32 PSUM matmuls vs host-built one-hot selector S (segment sum),
    ACT evacuation, indirect scatter-back to agg1 in DRAM.
  - Node phases (enc / upd / head) are small column-layout MLPs on PE.
"""

import math
import sys

import numpy as np

sys.path.insert(0, "/opt/trn_rl_repo")

import concourse.bass as bass
import concourse.mybir as mybir
import concourse.tile as tile
from concourse import bacc
from concourse.bass_utils import run_bass_kernel_spmd
from concourse.masks import make_identity

F32 = mybir.dt.float32
F16 = mybir.dt.float16
I32 = mybir.dt.int32

C = 8          # cores
GCH = 4        # chunks (of 128 edges) per group
GROUP_E = 128 * GCH   # 512 edges per group
WN = 32        # max distinct nodes per group == psum partition rows
BLK_G = 8      # groups per block
CPB = GCH * BLK_G     # chunks per block = 32
BLK_E = 128 * CPB     # edges per block = 4096


def _ceil_to(x, m):
    return ((x + m - 1) // m) * m


# ----------------------------------------------------------------------------
# Host preprocessing
# ----------------------------------------------------------------------------

def _preprocess(inputs):
    nf = np.asarray(inputs["node_features"], np.float32)
    edges = np.asarray(inputs["edges"])
    ef = np.asarray(inputs["edge_features"], np.float32)
    msg_w1 = np.asarray(inputs["msg_w1"], np.float32)
    msg_b1 = np.asarray(inputs["msg_b1"], np.float32)

    N, F = nf.shape
    E = edges.shape[0]
    H = msg_w1.shape[2]
    R = msg_w1.shape[0]
    assert H == 64, H

    NL = _ceil_to(N, C) // C              # real nodes per core (last may have fewer)
    NLP = _ceil_to(NL + 1, 128)           # padded; NLP-1 is a dummy scatter row
    NT = NLP // 128

    src = edges[:, 0].astype(np.int64)
    dst = edges[:, 1].astype(np.int64)
    core_of = np.minimum(dst // NL, C - 1)

    # zef[r] = ef @ W1c_r + b1_r   [E, H]
    zef_full = [(ef @ msg_w1[r, 2 * H:, :] + msg_b1[r]).astype(np.float32)
                for r in range(R)]

    def row_of(l):
        # p-major on-device row layout for [NLP, 64] tables
        return (l % 128) * NT + l // 128

    percore = []
    groups_per_core = []
    for c in range(C):
        e_sel = np.nonzero(core_of == c)[0]
        ld = dst[e_sel] - c * NL
        order = np.argsort(ld, kind="stable")
        eids = e_sel[order]
        ld_s = ld[order]

        uniq, counts = np.unique(ld_s, return_counts=True)
        starts = np.concatenate([[0], np.cumsum(counts)])
        groups = []  # (edge_id_array, col_array, node_array)
        gi = 0
        while gi < len(uniq):
            g_nodes = []
            g_e = 0
            g_start = starts[gi]
            while gi < len(uniq) and len(g_nodes) < WN and \
                    g_e + counts[gi] <= GROUP_E:
                g_nodes.append(uniq[gi])
                g_e += counts[gi]
                gi += 1
            assert g_nodes, f"single node degree exceeds {GROUP_E}"
            ge = eids[g_start:g_start + g_e]
            cols = np.repeat(np.arange(len(g_nodes)),
                             counts[gi - len(g_nodes):gi])
            groups.append((ge, cols, np.asarray(g_nodes)))
        groups_per_core.append(groups)

    NG = max(len(g) for g in groups_per_core)
    NG = _ceil_to(NG, BLK_G)
    NBLK = NG // BLK_G
    NCH = NG * GCH

    for c in range(C):
        groups = groups_per_core[c]
        slot = np.full((NCH, 128), -1, np.int64)
        colid = np.zeros((NCH, 128), np.int64)
        scat = np.full((NG, WN), NLP - 1, np.int64)
        for g, (ge, cols, gnodes) in enumerate(groups):
            k = len(ge)
            fl = np.full(GROUP_E, -1, np.int64)
            fc = np.zeros(GROUP_E, np.int64)
            fl[:k] = ge
            fc[:k] = cols
            slot[g * GCH:(g + 1) * GCH] = fl.reshape(GCH, 128)
            colid[g * GCH:(g + 1) * GCH] = fc.reshape(GCH, 128)
            scat[g, :len(gnodes)] = row_of(gnodes)

        valid = slot >= 0
        sv = np.where(valid, slot, 0)

        s_nodes = src[sv]
        aidx = (s_nodes // NL) * NLP + row_of(s_nodes % NL)
        aidx = np.where(valid, aidx, 0).astype(np.int32)
        b_nodes = dst[sv] - c * NL
        bidx = np.where(valid, row_of(b_nodes), 0).astype(np.int32)

        S = np.zeros((NCH, 128, WN), np.float32)
        ch_i, p_i = np.nonzero(valid)
        S[ch_i, p_i, colid[valid]] = 1.0

        def blockify(a):
            # [NCH, 128, ...] -> [NBLK, 128, CPB, ...]
            return np.ascontiguousarray(
                a.reshape((NBLK, CPB) + a.shape[1:]).swapaxes(1, 2))

        d = {
            "aidx": blockify(aidx),
            "bidx": blockify(bidx),
            "S": blockify(S),
            "sidx": np.ascontiguousarray(
                scat.reshape(NBLK, BLK_G, WN).swapaxes(1, 2)).astype(np.int32),
        }
        for r in range(R):
            z = np.zeros((NCH, 128, H), np.float16)
            z[valid] = zef_full[r][sv[valid]].astype(np.float16)
            d[f"zef{r}"] = blockify(z)

        NL_c = min(NL, N - c * NL)
        deg = np.zeros((1, NLP), np.float32)
        ln, cnt = np.unique(dst[sv[valid]] - c * NL, return_counts=True)
        deg[0, ln] = cnt
        d["deg"] = deg

        nfT = np.zeros((F, NLP), np.float32)
        nfT[:, :NL_c] = nf[c * NL:c * NL + NL_c].T
        d["nfT"] = nfT
        percore.append(d)

    meta = dict(N=N, E=E, F=F, H=H, R=R, NL=NL, NLP=NLP, NT=NT,
                NG=NG, NBLK=NBLK)
    return meta, percore


def _shared_weights(inputs, meta):
    R, H, F = meta["R"], meta["H"], meta["F"]
    g = lambda k: np.asarray(inputs[k], np.float32)
    w = {
        "enc_w1": g("enc_w1"), "enc_b1": g("enc_b1").reshape(H, 1),
        "enc_w2": g("enc_w2"), "enc_b2": g("enc_b2").reshape(H, 1),
        "head_w1": g("head_w1"), "head_b1": g("head_b1").reshape(H, 1),
        "head_w2": g("head_w2"), "head_b2": g("head_b2").reshape(1, 1),
    }
    msg_w1, msg_b2 = g("msg_w1"), g("msg_b2")
    w["w1a"] = np.ascontiguousarray(msg_w1[:, :H, :])
    w["w1b"] = np.ascontiguousarray(msg_w1[:, H:2 * H, :])
    w["w2ext"] = np.concatenate([g("msg_w2"), msg_b2[:, None, :]], axis=1)
    w["uw1"] = g("upd_w1")
    w["ub1"] = g("upd_b1").reshape(R, H, 1)
    w["uw2"] = g("upd_w2")
    w["ub2"] = g("upd_b2").reshape(R, H, 1)
    return w


# ----------------------------------------------------------------------------
# Device program
# ----------------------------------------------------------------------------

def _node_slices(NLP):
    out = []
    s = 0
    while s < NLP:
        w = min(512, NLP - s)
        out.append((s, w))
        s += w
    return out


def _build(meta):
    F, H, R = meta["F"], meta["H"], meta["R"]
    NLP, NT, NBLK = meta["NLP"], meta["NT"], meta["NBLK"]

    nc = bacc.Bacc("TRN2", target_bir_lowering=False, debug=False,
                   num_devices=C)

    dI = {}
    def din(name, shape, dt=F32):
        dI[name] = nc.dram_tensor(name, list(shape), dt,
                                  kind="ExternalInput").ap()

    din("nfT", (F, NLP))
    din("deg", (1, NLP))
    din("aidx", (NBLK, 128, CPB), I32)
    din("bidx", (NBLK, 128, CPB), I32)
    din("sidx", (NBLK, WN, BLK_G), I32)
    din("S", (NBLK, 128, CPB, WN))
    for r in range(R):
        din(f"zef{r}", (NBLK, 128, CPB, H), F16)
    din("enc_w1", (F, H)); din("enc_b1", (H, 1))
    din("enc_w2", (H, H)); din("enc_b2", (H, 1))
    din("head_w1", (H, H)); din("head_b1", (H, 1))
    din("head_w2", (H, 1)); din("head_b2", (1, 1))
    din("w1a", (R, H, H)); din("w1b", (R, H, H)); din("w2ext", (R, H + 1, H))
    din("uw1", (R, 2 * H, H)); din("ub1", (R, H, 1))
    din("uw2", (R, H, H)); din("ub2", (R, H, 1))

    out_d = nc.dram_tensor("out", [1, NLP], F32, kind="ExternalOutput").ap()

    acc = [nc.dram_tensor(f"acc{r}", [NLP, H], F32).ap() for r in range(R)]
    afull = [nc.dram_tensor(f"afull{r}", [C * NLP, H], F32,
                            addr_space="Shared").ap() for r in range(R)]
    bloc = [nc.dram_tensor(f"bloc{r}", [NLP, H], F32).ap() for r in range(R)]
    agg1 = [nc.dram_tensor(f"agg1_{r}", [NLP, H], F32).ap() for r in range(R)]

    NSL = _node_slices(NLP)
    rg = [list(range(C))]

    with tile.TileContext(nc) as tc:
        with (
            tc.tile_pool(name="cpool", bufs=1) as cpool,
            tc.tile_pool(name="npool", bufs=3) as npool,
            tc.tile_pool(name="epool", bufs=3) as epool,
            tc.tile_pool(name="psA", bufs=4, space="PSUM") as psA,
            tc.tile_pool(name="psN", bufs=3, space="PSUM") as psN,
        ):
            ident = cpool.tile([128, 128], dtype=F32)
            make_identity(nc, ident[:])

            # weights to SBUF
            wt = {}
            for nm in ("enc_w1", "enc_b1", "enc_w2", "enc_b2",
                       "head_w1", "head_b1", "head_w2", "head_b2"):
                wt[nm] = cpool.tile(list(dI[nm].shape), dtype=F32, tag=nm, name=nm)
                nc.sync.dma_start(out=wt[nm][:], in_=dI[nm][:, :])
            for nm in ("w1a", "w1b", "w2ext", "uw1", "ub1", "uw2", "ub2"):
                for r in range(R):
                    k = f"{nm}{r}"
                    shp = list(dI[nm].shape[1:])
                    wt[k] = cpool.tile(shp, dtype=F32, tag=k, name=k)
                    nc.sync.dma_start(out=wt[k][:], in_=dI[nm][r])

            upd_in = cpool.tile([2 * H, NLP], dtype=F32, tag="upd_in")
            rhs2 = cpool.tile([H + 1, NLP], dtype=F32, tag="rhs2")
            nc.sync.dma_start(out=rhs2[H:H + 1, :], in_=dI["deg"][:, :])

            # zero agg1 buffers
            zt = npool.tile([128, NT * H], dtype=F32, tag="zeros", bufs=1)
            nc.gpsimd.memset(zt[:], 0.0)
            for r in range(R):
                nc.sync.dma_start(
                    out=agg1[r].rearrange("(p q) f -> p (q f)", p=128),
                    in_=zt[:])

            # ---------------- node-phase helpers ----------------
            def mlp2_to(dst_ap_fn, w1, b1, w2, b2, rhs_fn, relu2=False):
                # dst[:, s:s+w] = (relu(w1.T @ rhs + b1)) @ w2 + b2
                for s, wd in NSL:
                    p1 = psN.tile([128, 512], dtype=F32, tag="nps")
                    nc.tensor.matmul(out=p1[:w1.shape[1], :wd], lhsT=w1[:],
                                     rhs=rhs_fn(s, wd), start=True, stop=True)
                    r1 = npool.tile([H, 512], dtype=F32, tag="r1")
                    nc.scalar.activation(
                        out=r1[:, :wd], in_=p1[:H, :wd],
                        func=mybir.ActivationFunctionType.Relu, bias=b1[:, :1])
                    p2 = psN.tile([128, 512], dtype=F32, tag="nps")
                    m2 = w2.shape[1]
                    nc.tensor.matmul(out=p2[:m2, :wd], lhsT=w2[:],
                                     rhs=r1[:, :wd], start=True, stop=True)
                    dst_ap_fn(s, wd, p2, m2, b2)

            def store_add_bias(dst_tile, row0):
                def f(s, wd, p2, m2, b2):
                    nc.vector.tensor_scalar_add(
                        dst_tile[row0:row0 + m2, s:s + wd],
                        p2[:m2, :wd], b2[:m2, :1])
                return f

            # encoder: h -> upd_in[0:H]
            nft = {}
            def nfT_rhs(s, wd):
                t = npool.tile([F, 512], dtype=F32, tag="nfsl")
                nc.sync.dma_start(out=t[:, :wd], in_=dI["nfT"][:, s:s + wd])
                return t[:, :wd]
            mlp2_to(store_add_bias(upd_in, 0), wt["enc_w1"], wt["enc_b1"],
                    wt["enc_w2"], wt["enc_b2"], nfT_rhs)

            def make_table(r, w_t, dram_rows, do_cc):
                # rows table = (h @ W).rows  (p-major layout), maybe AllGather
                rows = npool.tile([128, NT, H], dtype=F32, tag="rows128",
                                  bufs=2)
                for s, wd in NSL:
                    p1 = psN.tile([128, 512], dtype=F32, tag="nps")
                    nc.tensor.matmul(out=p1[:H, :wd], lhsT=w_t[:],
                                     rhs=upd_in[0:H, s:s + wd],
                                     start=True, stop=True)
                    sb = npool.tile([H, 512], dtype=F32, tag="absb")
                    nc.scalar.copy(out=sb[:, :wd], in_=p1[:H, :wd])
                    for q0 in range(wd // 128):
                        q = s // 128 + q0
                        pt = psN.tile([128, 512], dtype=F32, tag="nps")
                        nc.tensor.transpose(
                            out=pt[:128, :H],
                            in_=sb[:, q0 * 128:(q0 + 1) * 128],
                            identity=ident[:H, :H])
                        nc.scalar.copy(out=rows[:, q, :], in_=pt[:128, :H])
                nc.sync.dma_start(
                    out=dram_rows.rearrange("(p q) f -> p q f", p=128),
                    in_=rows[:])
                if do_cc:
                    nc.gpsimd.collective_compute(
                        "AllGather", mybir.AluOpType.bypass,
                        replica_groups=rg,
                        ins=[acc[r][:, :].opt()],
                        outs=[afull[r][:, :].opt()])

            make_table(0, wt["w1a0"], acc[0], True)
            make_table(0, wt["w1b0"], bloc[0], False)

            # ---------------- rounds ----------------
            for r in range(R):
                # edge phase
                for b in range(NBLK):
                    z = epool.tile([128, CPB, H], dtype=F32, tag="z")
                    nc.gpsimd.dma_start(out=z[:], in_=dI[f"zef{r}"][b])
                    ai = epool.tile([128, CPB], dtype=I32, tag="ai")
                    nc.sync.dma_start(out=ai[:], in_=dI["aidx"][b])
                    bi = epool.tile([128, CPB], dtype=I32, tag="bi")
                    nc.sync.dma_start(out=bi[:], in_=dI["bidx"][b])
                    st = epool.tile([128, CPB, WN], dtype=F32, tag="st")
                    nc.sync.dma_start(out=st[:], in_=dI["S"][b])
                    for ch in range(CPB):
                        nc.gpsimd.indirect_dma_start(
                            out=z[:, ch, :], out_offset=None,
                            in_=afull[r][:, :],
                            in_offset=bass.IndirectOffsetOnAxis(
                                ap=ai[:, ch:ch + 1], axis=0),
                            compute_op=mybir.AluOpType.add)
                    for ch in range(CPB):
                        nc.gpsimd.indirect_dma_start(
                            out=z[:, ch, :], out_offset=None,
                            in_=bloc[r][:, :],
                            in_offset=bass.IndirectOffsetOnAxis(
                                ap=bi[:, ch:ch + 1], axis=0),
                            compute_op=mybir.AluOpType.add)
                    u = epool.tile([128, CPB, H], dtype=F32, tag="u")
                    nc.vector.tensor_scalar_max(
                        u[:].rearrange("p a b -> p (a b)"),
                        z[:].rearrange("p a b -> p (a b)"), 0.0)
                    ev = epool.tile([WN, BLK_G, H], dtype=F32, tag="ev")
                    for g in range(BLK_G):
                        ps = psA.tile([WN, H], dtype=F32, tag="aps")
                        for cc in range(GCH):
                            ch = g * GCH + cc
                            nc.tensor.matmul(out=ps[:], lhsT=st[:, ch, :],
                                             rhs=u[:, ch, :],
                                             start=(cc == 0),
                                             stop=(cc == GCH - 1))
                        nc.scalar.copy(out=ev[:, g, :], in_=ps[:])
                    si = epool.tile([WN, BLK_G], dtype=I32, tag="si")
                    nc.sync.dma_start(out=si[:], in_=dI["sidx"][b])
                    for g in range(BLK_G):
                        nc.gpsimd.indirect_dma_start(
                            out=agg1[r][:, :],
                            out_offset=bass.IndirectOffsetOnAxis(
                                ap=si[:, g:g + 1], axis=0),
                            in_=ev[:, g, :], in_offset=None)

                # node phase: read agg1 back, transpose, W2ext, update MLP
                rows = npool.tile([128, NT, H], dtype=F32, tag="rows128",
                                  bufs=2)
                nc.sync.dma_start(
                    out=rows[:],
                    in_=agg1[r].rearrange("(p q) f -> p q f", p=128))
                for q in range(NT):
                    pt = psN.tile([128, 512], dtype=F32, tag="nps")
                    nc.tensor.transpose(out=pt[:H, :128], in_=rows[:, q, :],
                                        identity=ident[:, :])
                    nc.scalar.copy(out=rhs2[0:H, q * 128:(q + 1) * 128],
                                   in_=pt[:H, :128])
                # agg final = w2ext.T @ rhs2  -> upd_in[H:2H]
                for s, wd in NSL:
                    p1 = psN.tile([128, 512], dtype=F32, tag="nps")
                    nc.tensor.matmul(out=p1[:H, :wd], lhsT=wt[f"w2ext{r}"][:],
                                     rhs=rhs2[:, s:s + wd],
                                     start=True, stop=True)
                    nc.scalar.copy(out=upd_in[H:2 * H, s:s + wd],
                                   in_=p1[:H, :wd])
                # update MLP: h += relu(uw1.T @ [h;agg] + ub1) @ uw2 + ub2
                for s, wd in NSL:
                    p1 = psN.tile([128, 512], dtype=F32, tag="nps")
                    nc.tensor.matmul(out=p1[:H, :wd], lhsT=wt[f"uw1{r}"][:],
                                     rhs=upd_in[:, s:s + wd],
                                     start=True, stop=True)
                    r1 = npool.tile([H, 512], dtype=F32, tag="r1")
                    nc.scalar.activation(
                        out=r1[:, :wd], in_=p1[:H, :wd],
                        func=mybir.ActivationFunctionType.Relu,
                        bias=wt[f"ub1{r}"][:, :1])
                    p2 = psN.tile([128, 512], dtype=F32, tag="nps")
                    nc.tensor.matmul(out=p2[:H, :wd], lhsT=wt[f"uw2{r}"][:],
                                     rhs=r1[:, :wd], start=True, stop=True)
                    t2 = npool.tile([H, 512], dtype=F32, tag="t2")
                    nc.vector.tensor_scalar_add(t2[:, :wd], p2[:H, :wd],
                                                wt[f"ub2{r}"][:, :1])
                    nc.vector.tensor_add(upd_in[0:H, s:s + wd],
                                         upd_in[0:H, s:s + wd], t2[:, :wd])

                if r + 1 < R:
                    make_table(r + 1, wt[f"w1a{r + 1}"], acc[r + 1], True)
                    make_table(r + 1, wt[f"w1b{r + 1}"], bloc[r + 1], False)

            # head -> out
            for s, wd in NSL:
                p1 = psN.tile([128, 512], dtype=F32, tag="nps")
                nc.tensor.matmul(out=p1[:H, :wd], lhsT=wt["head_w1"][:],
                                 rhs=upd_in[0:H, s:s + wd],
                                 start=True, stop=True)
                r1 = npool.tile([H, 512], dtype=F32, tag="r1")
                nc.scalar.activation(
                    out=r1[:, :wd], in_=p1[:H, :wd],
                    func=mybir.ActivationFunctionType.Relu,
                    bias=wt["head_b1"][:, :1])
                p2 = psN.tile([128, 512], dtype=F32, tag="nps")
                nc.tensor.matmul(out=p2[:1, :wd], lhsT=wt["head_w2"][:, :1],
                                 rhs=r1[:, :wd], start=True, stop=True)
                o = npool.tile([1, 512], dtype=F32, tag="osl")
                nc.vector.tensor_scalar_add(o[:1, :wd], p2[:1, :wd],
                                            wt["head_b2"][:1, :1])
                nc.sync.dma_start(out=out_d[:, s:s + wd], in_=o[:1, :wd])

    nc.compile()
    return nc


# ----------------------------------------------------------------------------
# Entry point
# ----------------------------------------------------------------------------

_CACHE = {}


def _prep_and_build(inputs):
    meta, percore = _preprocess(inputs)
    w = _shared_weights(inputs, meta)
    key = (meta["N"], meta["E"], meta["F"], meta["H"], meta["R"],
           meta["NBLK"])
    if key not in _CACHE:
        _CACHE[key] = _build(meta)
    nc = _CACHE[key]

    in_maps = []
    for c in range(C):
        m = dict(percore[c])
        m.update({k: np.ascontiguousarray(v) for k, v in w.items()})
        in_maps.append(m)
    return meta, nc, in_maps


def kernel(trace=False, **inputs):
    meta, nc, in_maps = _prep_and_build(inputs)
    res = run_bass_kernel_spmd(nc, in_maps, core_ids=list(range(C)),
                               trace=trace)
    N, NL = meta["N"], meta["NL"]
    parts = []
    for c in range(C):
        NL_c = min(NL, N - c * NL)
        parts.append(np.asarray(res.results[c]["out"])[0, :NL_c])
    out = np.concatenate(parts).astype(np.float32)
    kernel.last_exec_time_ns = res.exec_time_ns
    return out


kernel.last_exec_time_ns = None
